# revision 54
# baseline (speedup 1.0000x reference)
"""Trainium2 Bass kernel for nn_CrossAttention_55130200212194.

Sharding: head h -> core h (8 heads, 8 cores, one replicated NEFF; cores
differ only in input data).  Host prep = layout/dtype only (transposes,
bf16 casts, constant prescales); every FLOP of the module runs on device.
Host combine = sum of the 8 partial [2048,640] projections (column-
sharded Wout, bf16 partials).

All-bf16 data path.  fp8/DoubleRow and Schraudolph-exp variants were
measured and rejected: the output is an attention-weighted average, so
multiplicative quantization noise passes through at full relative
strength (any single fp8 stage costs ~2e-2 max-rel-err vs the 2e-2
gate; bf16 lands at 4.2e-3 end to end).

  - scores: 3 bf16 matmuls per j-tile (K=80; bf16 runs 1 PE cyc/row at
    any free size), gamma/SCALE folded into the host k prescales, all
    three k tensors packed in one DMA tensor.
  - exp: exact, ACT-only (the 66us floor of this kernel), one op per
    j-tile over a 2-bank PSUM pair: mix+self paths share one [P,2,IC]
    tile and one shift of -4.5 (self logits reach 8.47; the shift
    cancels in the softmax ratio) -> bf16 em/es pair tile.
  - attn@v in [i,d] output orientation: em/es [j,i] slices are the
    STATIONARY operand (weight loads are free in the cost model), v
    extended to 96 cols (col 80 = 1/gamma resp. 1/beta) is the moving
    operand: 4x96 cyc per j-tile instead of 2x512.  Z/gamma lands as a
    per-partition COLUMN; 1/Z via single-op reciprocal_approx_fast;
    normalization is two per-partition-scalar DVE ops (no broadcast
    matmuls, no one-hot weights).
  - merged [i,d] tiles (128 cols, col 96 = 1.0) transpose to [d,i] via
    DMA-crossbar mid-loop (HWDGE is idle there) and via PE is_transpose
    matmuls at the tail (skips the 625ns HWDGE + 900ns DMA-sem chain);
    either way the transpose plants the ones-row at row 96 that pairs
    with WoT's bias row (core 0 only), so the projection bias is free.
  - software pipelining: attn@v lags scores/exp by one j-tile across
    chunk boundaries; qc/v_self prologue GEMMs and output projections
    interleave into the j-loop on a shared 2-bank PSUM tag, so ACT
    starts at ~6us with no bulk prologue; the last chunk's merge/
    project/evac chain is stage-ordered to pipeline across DVE/PE/ACT.
  - outputs stored bf16 (halves output DMA bytes; host sums as f32).
"""

import os
import sys

sys.path.insert(0, "/opt/trn_rl_repo")

import numpy as np
import ml_dtypes

H = 8
N = 2048
D = 80
C = 640
SCALE = D ** -0.5
GAMMA = 0.7
BETA = 0.3
P = 128
IC = 512                 # i-chunk (PSUM bank = 512 fp32)
NJT = N // P             # 16 j-tiles
NICH = N // IC           # 4 i-chunks
NSUB = IC // P           # 4 i-subtiles per chunk
VE = 96                  # v extended cols: 80 d + 1/w col + zeros
NCORES = 8

SHIFT = 4.5              # unified logit shift (self logits reach 8.47)

BF16 = ml_dtypes.bfloat16

_CACHE = {}
LAST_EXEC_NS = None


def _build_nc():
    import concourse.mybir as mybir
    import concourse.tile as tile
    from concourse import bacc
    from concourse.bass import ts

    f32 = mybir.dt.float32
    bf16 = mybir.dt.bfloat16
    Exp = mybir.ActivationFunctionType.Exp
    Alu = mybir.AluOpType

    nc = bacc.Bacc(
        "TRN2",
        target_bir_lowering=False,
        debug=False,
        enable_asserts=False,
        num_devices=NCORES,
    )

    xT_d = nc.dram_tensor("xT", [P, 5, N], bf16, kind="ExternalInput")
    qiT_d = nc.dram_tensor("qiT", [D, N], bf16, kind="ExternalInput")
    k3_d = nc.dram_tensor("k3", [D, 3, N], bf16, kind="ExternalInput")
    ve_d = nc.dram_tensor("ve", [P, NJT, VE], bf16, kind="ExternalInput")
    WqT_d = nc.dram_tensor("WqT", [P, 5, D], bf16, kind="ExternalInput")
    WvT_d = nc.dram_tensor("WvT", [P, 5, D], bf16, kind="ExternalInput")
    WoT_d = nc.dram_tensor("WoT", [P, C], bf16, kind="ExternalInput")
    ident_d = nc.dram_tensor("ident", [P, P], bf16, kind="ExternalInput")
    out_d = nc.dram_tensor("out", [N, C], bf16, kind="ExternalOutput")

    with tile.TileContext(nc) as tc:
        with (
            tc.tile_pool(name="const", bufs=1) as const,
            tc.tile_pool(name="work", bufs=2) as work,
            tc.tile_pool(name="fout", bufs=3) as fout,
            tc.tile_pool(name="psum", bufs=1, space="PSUM") as pm,
        ):
            xT = const.tile([P, 5, N], bf16, tag="xT")
            qiT = const.tile([P, N], bf16, tag="qiT")
            qcT = const.tile([P, N], bf16, tag="qcT")
            k3 = const.tile([P, 3, N], bf16, tag="k3")
            v_e = const.tile([P, NJT, VE], bf16, tag="v_e")
            vs_e = const.tile([P, NJT, VE], bf16, tag="vs_e")
            WqT = const.tile([P, 5, D], bf16, tag="WqT")
            WvT = const.tile([P, 5, D], bf16, tag="WvT")
            WoT = const.tile([P, C], bf16, tag="WoT")
            mergedT = const.tile([P, N], bf16, tag="mergedT")
            # merged [i,d] staging: 8 slots (chunk parity x 4 i-subs),
            # cols 80:128 zero except col 96 = 1.0: the DMA transpose
            # plants mergedT's ones bias-row (96) + zero rows for free.
            mg = const.tile([P, 2 * NSUB, P], bf16, tag="mg")
            ident = const.tile([P, P], bf16, tag="ident")
            nbias = const.tile([P, 1], f32, tag="nbias")

            nc.gpsimd.memset(nbias[:], -SHIFT)
            nc.gpsimd.memset(mg[:], 0.0)
            nc.gpsimd.memset(mg[:, :, 96:97], 1.0)
            nc.gpsimd.memset(vs_e[:, :, D:VE], 0.0)
            nc.gpsimd.memset(vs_e[:, :, D : D + 1], 1.0 / BETA)
            # prefetch the Exp table during the input DMA window
            nc.scalar.activation(
                nbias[0:1, 0:1],
                nbias[0:1, 0:1],
                Exp,
                bias=nbias[0:1, 0:1],
                scale=0.0,
            )
            nc.gpsimd.memset(nbias[:], -SHIFT)

            # ---- DMAs in consumer-priority order (first-use order; the
            # sim serializes all queues on one DMA device token) ----
            nc.sync.dma_start(WqT[:], WqT_d.ap())
            nc.sync.dma_start(xT[:, :, ts(0, IC)], xT_d.ap()[:, :, ts(0, IC)])
            nc.sync.dma_start(qiT[0:D, ts(0, IC)], qiT_d.ap()[:, ts(0, IC)])
            # all three k tensors in one tensor; j-tiles 0-1 first
            nc.sync.dma_start(k3[0:D, :, 0:256], k3_d.ap()[:, :, 0:256])
            nc.sync.dma_start(WvT[:], WvT_d.ap())
            nc.sync.dma_start(v_e[:], ve_d.ap())
            nc.sync.dma_start(k3[0:D, :, 256:N], k3_d.ap()[:, :, 256:N])
            nc.sync.dma_start(qiT[0:D, IC:N], qiT_d.ap()[:, IC:N])
            for ic in range(1, NICH):
                nc.sync.dma_start(
                    xT[:, :, ts(ic, IC)], xT_d.ap()[:, :, ts(ic, IC)]
                )
            nc.sync.dma_start(WoT[:], WoT_d.ap())
            nc.sync.dma_start(ident[:], ident_d.ap())

            # shared 2-bank aux tag for qc / v_self / projections
            def aux_tile():
                return pm.tile([P, 2 * IC], f32, tag="fin", bufs=1,
                               name="aux")

            def qc_block(ic):
                qps = aux_tile()
                for c in range(5):
                    nc.tensor.matmul(
                        qps[0:D, 0:IC], WqT[:, c, :], xT[:, c, ts(ic, IC)],
                        start=(c == 0), stop=(c == 4),
                        skip_group_check=True,
                    )
                nc.vector.tensor_copy(qcT[0:D, ts(ic, IC)], qps[0:D, 0:IC])

            def vself_quad(q):
                # n-tiles 4q..4q+3, 256-fp32 slots; bank starts at k 0 / 2
                psv = aux_tile()
                for k in range(4):
                    t = 4 * q + k
                    for c in range(5):
                        nc.tensor.matmul(
                            psv[:, k * 256 : k * 256 + D],
                            xT[:, c, ts(t, P)], WvT[:, c, :],
                            start=(c == 0 and k % 2 == 0),
                            stop=(c == 4),
                            skip_group_check=True,
                        )
                for k in range(4):
                    t = 4 * q + k
                    nc.vector.tensor_copy(
                        vs_e[:, t, 0:D], psv[:, k * 256 : k * 256 + D]
                    )

            def project(pic, t):
                nt = 4 * pic + t
                fin = aux_tile()
                nc.tensor.matmul(
                    fin[:, 0:IC], mergedT[:, ts(nt, P)], WoT[:, 0:IC],
                    start=True, stop=True, skip_group_check=True,
                )
                nc.tensor.matmul(
                    fin[:, IC:C], mergedT[:, ts(nt, P)], WoT[:, IC:C],
                    start=True, stop=True, skip_group_check=True,
                )
                fsb = fout.tile([P, C], bf16, tag="fsb", bufs=4)
                nc.vector.tensor_copy(fsb[:], fin[:, 0:C])
                nc.sync.dma_start(out_d.ap()[ts(nt, P), :], fsb[:])

            def merge_isub(pic, oDS, s):
                slot = (pic % 2) * NSUB + s
                rcol = work.tile([P, 2], f32, tag="rc", bufs=8)
                nc.vector.reciprocal_approx_fast(
                    out=rcol[:, 0:1], in_=oDS[:, 0, s, D : D + 1]
                )
                nc.vector.reciprocal_approx_fast(
                    out=rcol[:, 1:2], in_=oDS[:, 1, s, D : D + 1]
                )
                nc.vector.tensor_scalar(
                    mg[:, slot, 0:D], oDS[:, 0, s, 0:D],
                    rcol[:, 0:1], None, Alu.mult,
                )
                nc.vector.scalar_tensor_tensor(
                    mg[:, slot, 0:D], oDS[:, 1, s, 0:D],
                    rcol[:, 1:2], mg[:, slot, 0:D],
                    Alu.mult, Alu.add,
                )
                nc.sync.dma_start_transpose(
                    mergedT[:, ts(NSUB * pic + s, P)], mg[:, slot, :]
                )

            # ---- fused prologue + main loop, software-pipelined ----
            qc_block(0)

            pend = []            # (emes, j, oDS) pending attn@v, depth 1
            for ic in range(NICH):
                win = ts(ic, IC)
                # [P, slot(D/S), isub, 128]: slot0 = bank0, slot1 = bank1
                oDS = pm.tile([P, 2, NSUB, P], f32, tag="o", bufs=1)
                for j in range(NJT):
                    sc = pm.tile([P, 2, IC], f32, tag="sc", bufs=2)
                    nc.tensor.matmul(
                        sc[:, 0, :], k3[0:D, 0, ts(j, P)], qiT[0:D, win],
                        start=True, stop=False,
                    )
                    nc.tensor.matmul(
                        sc[:, 0, :], k3[0:D, 2, ts(j, P)], qcT[0:D, win],
                        start=False, stop=True,
                    )
                    nc.tensor.matmul(
                        sc[:, 1, :], k3[0:D, 1, ts(j, P)], qiT[0:D, win],
                        start=True, stop=True,
                    )
                    emes = work.tile([P, 2, IC], bf16, tag="e", bufs=4)
                    nc.scalar.activation(
                        emes[:], sc[:], Exp, bias=nbias[:, 0:1], scale=1.0,
                    )
                    # deferred attn@v, one j-tile behind (cross-chunk)
                    if len(pend) >= 1:
                        pem, pj, poDS = pend.pop(0)
                        for s in range(NSUB):
                            nc.tensor.matmul(
                                poDS[:, 0, s, 0:VE], pem[:, 0, ts(s, P)],
                                v_e[:, pj, :],
                                start=(pj == 0 and s == 0),
                                stop=(pj == NJT - 1),
                                skip_group_check=True,
                            )
                            nc.tensor.matmul(
                                poDS[:, 1, s, 0:VE], pem[:, 1, ts(s, P)],
                                vs_e[:, pj, :],
                                start=(pj == 0 and s == 0),
                                stop=(pj == NJT - 1),
                                skip_group_check=True,
                            )
                        if pj == NJT - 1:
                            # previous chunk complete: normalize + merge +
                            # transpose per i-subtile, then free its banks
                            for s in range(NSUB):
                                merge_isub(ic - 1, poDS, s)
                    pend.append((emes, j, oDS))
                    # interleaved prologue/projection work on the aux tag
                    if ic == 0:
                        if j == 0:
                            vself_quad(0)
                        elif j in (3, 7, 11):
                            vself_quad((j + 1) // 4)
                        elif j == 13:
                            qc_block(1)
                    else:
                        if j in (4, 7, 10, 13):
                            project(ic - 1, (4, 7, 10, 13).index(j))
                        elif j == 14 and ic < NICH - 1:
                            qc_block(ic + 1)
                if ic == NICH - 1:
                    # flush the two pending attn@v j-tiles
                    for pem, pj, poDS in pend:
                        for s in range(NSUB):
                            nc.tensor.matmul(
                                poDS[:, 0, s, 0:VE], pem[:, 0, ts(s, P)],
                                v_e[:, pj, :],
                                start=False, stop=(pj == NJT - 1),
                                skip_group_check=True,
                            )
                            nc.tensor.matmul(
                                poDS[:, 1, s, 0:VE], pem[:, 1, ts(s, P)],
                                vs_e[:, pj, :],
                                start=False, stop=(pj == NJT - 1),
                                skip_group_check=True,
                            )
                    pend = []
                    # merges: one batched 8-way reciprocal, then per-s
                    # normalize on DVE, PE transpose, DVE evac
                    rc8 = work.tile([P, 2, NSUB], f32, tag="rc8", bufs=1)
                    nc.vector.reciprocal_approx_fast(
                        out=rc8[:, :, :], in_=oDS[:, :, :, D]
                    )
                    for s in range(NSUB):
                        slot = (ic % 2) * NSUB + s
                        nc.vector.tensor_scalar(
                            mg[:, slot, 0:D], oDS[:, 0, s, 0:D],
                            rc8[:, 0, s : s + 1], None, Alu.mult,
                        )
                        nc.vector.scalar_tensor_tensor(
                            mg[:, slot, 0:D], oDS[:, 1, s, 0:D],
                            rc8[:, 1, s : s + 1], mg[:, slot, 0:D],
                            Alu.mult, Alu.add,
                        )
                        tal = aux_tile()
                        treg = tal[:, 768:832].bitcast(bf16)
                        nc.tensor.matmul(
                            treg, mg[:, slot, :], ident[:],
                            start=True, stop=True, is_transpose=True,
                            skip_group_check=True,
                        )
                        nc.vector.tensor_copy(
                            mergedT[:, ts(NSUB * ic + s, P)], treg
                        )
                    # tail projections pipelined; evacs split DVE/ACT
                    fins = []
                    for s in range(NSUB):
                        nt = 4 * ic + s
                        fin = aux_tile()
                        nc.tensor.matmul(
                            fin[:, 0:IC], mergedT[:, ts(nt, P)], WoT[:, 0:IC],
                            start=True, stop=True, skip_group_check=True,
                        )
                        nc.tensor.matmul(
                            fin[:, IC:C], mergedT[:, ts(nt, P)],
                            WoT[:, IC:C],
                            start=True, stop=True, skip_group_check=True,
                        )
                        fins.append(fin)
                        if s >= 1:
                            pf = fins[s - 1]
                            nt0 = 4 * ic + s - 1
                            fsb = fout.tile([P, C], bf16, tag="fsb", bufs=4)
                            if s % 2 == 1:
                                nc.vector.tensor_copy(fsb[:], pf[:, 0:C])
                            else:
                                nc.scalar.copy(fsb[:], pf[:, 0:C])
                            nc.sync.dma_start(out_d.ap()[ts(nt0, P), :], fsb[:])
                    fsb = fout.tile([P, C], bf16, tag="fsb", bufs=4)
                    nc.scalar.copy(fsb[:], fins[3][:, 0:C])
                    nc.sync.dma_start(out_d.ap()[ts(4 * ic + 3, P), :], fsb[:])

    nc.compile()
    return nc


def _get_nc():
    if "nc" not in _CACHE:
        _CACHE["nc"] = _build_nc()
    return _CACHE["nc"]


def _prep_core(h, x, q_inj, k_inj, k_ref, k_refL, v_ref, Wq, Wv, Wout, bout):
    sl = slice(h * D, (h + 1) * D)

    xT = np.ascontiguousarray(x[0].T).reshape(5, P, N).transpose(1, 0, 2)

    ve = np.zeros((P, NJT, VE), BF16)
    ve[:, :, 0:D] = v_ref[h].reshape(NJT, P, D).transpose(1, 0, 2).astype(BF16)
    ve[:, :, D] = np.float32(1.0 / GAMMA)

    WoT = np.zeros((P, C), BF16)
    WoT[0:D, :] = Wout[:, sl].T.astype(BF16)
    if h == 0:
        WoT[96, :] = bout.astype(BF16)

    return {
        "xT": np.ascontiguousarray(xT).astype(BF16),
        "qiT": np.ascontiguousarray(q_inj[h].T).astype(BF16),
        "k3": np.ascontiguousarray(np.stack([
            k_refL[h].T * (GAMMA * SCALE),
            k_inj[h].T * SCALE,
            k_ref[h].T * ((1.0 - GAMMA) * SCALE),
        ], axis=1)).astype(BF16),
        "ve": ve,
        "WqT": np.ascontiguousarray(
            Wq[sl, :].T.reshape(5, P, D).transpose(1, 0, 2)).astype(BF16),
        "WvT": np.ascontiguousarray(
            Wv[sl, :].T.reshape(5, P, D).transpose(1, 0, 2)).astype(BF16),
        "WoT": WoT,
        "ident": np.eye(P, dtype=np.float32).astype(BF16),
    }


def kernel(x, q_inj, k_inj, k_ref, k_refL, v_ref, Wq, Wv, Wout, bout):
    global LAST_EXEC_NS
    f = np.float32
    args = [np.asarray(a, f) for a in
            (x, q_inj, k_inj, k_ref, k_refL, v_ref, Wq, Wv, Wout, bout)]

    nc = _get_nc()
    in_maps = [_prep_core(h, *args) for h in range(NCORES)]

    from concourse.bass_utils import run_bass_kernel_spmd

    trace = bool(os.environ.get("TRN_TRACE"))
    try:
        res = run_bass_kernel_spmd(
            nc, in_maps, core_ids=list(range(NCORES)), trace=trace
        )
    except ModuleNotFoundError:
        res = run_bass_kernel_spmd(
            nc, in_maps, core_ids=list(range(NCORES)), trace=False
        )
    LAST_EXEC_NS = res.exec_time_ns
    out = np.zeros((N, C), f)
    for r in res.results:
        out += np.asarray(r["out"], f)
    return out.reshape(1, N, C)
